# revision 18
# baseline (speedup 1.0000x reference)
"""Trainium2 Bass kernel for the CRAFT-style hard-negative-mining MSE loss.

Reference math (per branch, over N = 16*768*768 flat pixels):
    all_loss = (pred - target)^2
    pos_mask = (target >= 0.3) & (weight != 0)
    neg_mask = (target < 0.1)
    pos_sum  = sum(pos_mask * all_loss * weight)
    k        = min(max(1000, 3*num_pos), num_neg)
    topk_sum = sum of k largest all_loss among negatives
    loss     = (pos_sum + topk_sum) / (num_pos + k)
    out      = loss_char + loss_aff

With uniform targets num_pos ~ 0.7*N, so 3*num_pos >> num_neg and
k == num_neg: the top-k degenerates to the full sum over negatives.

Device strategy (v7): per 1/8 shard, per branch-tile [128, W=2304],
work is spread so DVE / ScalarE / TensorE / DMA all sit just under the
~34us DMA roofline:
    DVE:  d  = p - t                    tensor_tensor        (2x mode)
          m_pos = (t >= 0.3)           tensor_scalar is_ge  (4x mode)
          mw = m_pos * w                tensor_tensor        (2x mode)
    ACT:  s_neg = sign(0.1 - t)  (+-1)  Sign                 (1x)
          l  = d^2                      Square               (1x)
    PE:   nb = W/96 blocks of 96 data cols:
            psum[0:97, 0:289] += [l_96 | 1]^T @ [s_neg | m_pos | mw | 1]
          accumulated over the whole branch into one [97, 289] PSUM
          region:
            diag(rows 0:96, group 0) -> <s_neg, l> = 2*S1 - sum(l)
            diag(rows 0:96, group 2) -> <mw,    l> = S2
            row 96, group 0          -> sum(s_neg) = 2*num_neg - N
            row 96, group 1          -> sum(m_pos) = num_pos
            col 288 (ones moving)    -> per-row partition sums of l,
                                        totalling sum(l)
The [97, 290] PSUM regions are copied to SBUF (ScalarE) and DMA'd out
in 4 partition slices over 4 DMA queues; the host extracts
diagonals/count-rows, recovers S1 = (<s_neg,l> + sum(l))/2 and
num_neg = (sum(s_neg) + N)/2, sums across the 8 shards, and applies
the k/denominator logic (with a full numpy fallback for the
never-hit-here k < num_neg case).

Inputs are cast to bf16 on the host: halves HBM traffic and doubles
DVE tensor_tensor throughput.
"""

import os
import numpy as np
import ml_dtypes

N_CORES = 8
B, H, W = 16, 768, 768
NPX = B * H * W              # 9_437_184 flat pixels
P = 128                      # SBUF partitions
FD = NPX // (N_CORES * P)    # 9216 free-dim elements per core per tensor
N_TILES = 4                  # tiles per branch
TW = FD // N_TILES           # 2304 tile width
BD = 96                      # data columns per matmul block
PR = BD + 1                  # psum rows used (96 data + 1 count row)
NB = TW // BD                # 24 matmul blocks per tile
MC = 3 * BD + 2              # moving row: s_neg | m_pos | mw | one | pad

THRESH_NEG = 0.1
THRESH_POS = 0.3

_compiled = None             # cached nc
LAST_RESULTS = None          # BassKernelResults of the last run (for profiling)


def _build_nc():
    import concourse.bacc as bacc
    import concourse.mybir as mybir
    import concourse.tile as tile
    from contextlib import ExitStack

    DT = mybir.dt.bfloat16
    f32 = mybir.dt.float32
    Alu = mybir.AluOpType
    Act = mybir.ActivationFunctionType

    nc = bacc.Bacc(
        "TRN2",
        target_bir_lowering=False,
        debug=False,
        num_devices=N_CORES,
    )

    # packed input: [P, branch, tile, (p,t,w), TW]
    pk = nc.declare_dram_parameter("pk", [P, 2, N_TILES, 3, TW], DT, isOutput=False)
    out_ps = nc.declare_dram_parameter("acc_ps", [PR, 2, MC - 1], f32, isOutput=True)

    with tile.TileContext(nc) as tc, ExitStack() as ctx:
        in_pool = ctx.enter_context(tc.tile_pool(name="in", bufs=3))
        d_pool = ctx.enter_context(tc.tile_pool(name="d", bufs=2))
        acc_pool = ctx.enter_context(tc.tile_pool(name="acc", bufs=1))
        ps_pool = ctx.enter_context(tc.psum_pool(name="ps", bufs=1))

        ps_sb = acc_pool.tile([PR, 2, MC - 1], f32, tag="ps_sb")
        psum = [
            ps_pool.tile([PR, MC - 1], f32, tag=f"psum{b}", name=f"psum{b}")
            for b in range(2)
        ]
        # per-partition f32 bias constant 0.1 for sign(0.1 - t)
        bias_neg = acc_pool.tile([P, 1], f32, tag="bias_neg")
        nc.gpsimd.memset(bias_neg[:], THRESH_NEG)
        # persistent double-buffered stationary [l_96 | 1] blocks; ones
        # column written once
        lexts = [
            acc_pool.tile([P, NB, PR], DT, tag=f"lext{j}", name=f"lext{j}")
            for j in range(2)
        ]
        # persistent double-buffered moving tensors [nb, s|m|mw|1|pad];
        # ones column written once, pad never touched
        ms = [
            acc_pool.tile([P, NB, MC], DT, tag=f"m{j}", name=f"m{j}")
            for j in range(2)
        ]
        for j in range(2):
            nc.gpsimd.memset(lexts[j][:, :, BD : BD + 1], 1.0)
            nc.gpsimd.memset(ms[j][:, :, 3 * BD : 3 * BD + 1], 1.0)

        it = 0
        for b in range(2):
            for i in range(N_TILES):
                tin = in_pool.tile([P, 3, TW], DT, tag="in")
                nc.sync.dma_start(tin[:], pk[:, b, i])
                pt = tin[:, 0, :]
                tt = tin[:, 1, :]
                wt = tin[:, 2, :]

                lext = lexts[it % 2]
                m = ms[it % 2]
                # s_neg = sign(0.1 - t), +-1 exactly        (ACT Sign 1x)
                nc.scalar.activation(
                    m[:, :, 0:BD], tt, Act.Sign, bias=bias_neg[:], scale=-1.0
                )
                # d = pred - target                          (DVE TT 2x)
                d = d_pool.tile([P, TW], DT, tag="d")
                nc.vector.tensor_tensor(d[:], pt, tt, Alu.subtract)
                # l = d^2 into cols 0:96 of the 97-blocks    (ACT Square 1x)
                nc.scalar.activation(lext[:, :, 0:BD], d[:], Act.Square)
                # m_pos = (t >= 0.3)                         (DVE TS 4x)
                nc.vector.tensor_scalar(
                    m[:, :, BD : 2 * BD], tt, THRESH_POS, None, Alu.is_ge
                )
                # mw = m_pos * w                             (DVE TT 2x)
                nc.vector.tensor_tensor(
                    m[:, :, 2 * BD : 3 * BD],
                    m[:, :, BD : 2 * BD],
                    wt,
                    Alu.mult,
                )

                # psum[b] += [l_blk | 1]^T @ [s_neg | m_pos | mw | 1]  (PE)
                for k in range(NB):
                    nc.tensor.matmul(
                        psum[b][:, :],
                        lext[:, k, :],
                        m[:, k, 0 : MC - 1],
                        start=(i == 0 and k == 0),
                        stop=(i == N_TILES - 1 and k == NB - 1),
                    )
                it += 1

            # dump the accumulated [97, 289] PSUM region to SBUF (ScalarE),
            # then DMA it out in 4 partition slices on 4 different engine
            # queues (a single contiguous store serializes on one DMA ring)
            nc.scalar.copy(ps_sb[:, b], psum[b][:, :])
            slices = [(0, 25), (25, 49), (49, 73), (73, PR)]
            issuers = [nc.sync, nc.gpsimd, nc.scalar, nc.gpsimd]
            for (p0, p1), eng in zip(slices, issuers):
                eng.dma_start(out_ps[p0:p1, b], ps_sb[p0:p1, b])

    nc.compile()
    return nc


def _get_nc():
    global _compiled
    if _compiled is None:
        _compiled = _build_nc()
    return _compiled


def _np_branch_fallback(pred, target, weight):
    """Exact reference math in numpy float64 (handles k < num_neg)."""
    pred = pred.astype(np.float64)
    target = target.astype(np.float64)
    weight = weight.astype(np.float64)
    all_loss = (pred - target) ** 2
    pos_mask = (target >= THRESH_POS) & (weight != 0)
    neg_mask = target < THRESH_NEG
    pos_sum = float(np.sum(np.where(pos_mask, all_loss * weight, 0.0)))
    num_pos = int(np.sum(pos_mask))
    num_neg = int(np.sum(neg_mask))
    k = min(max(1000, 3 * num_pos), num_neg)
    neg_vals = all_loss[neg_mask]
    if k >= num_neg:
        topk = float(neg_vals.sum())
    elif k <= 0:
        topk = 0.0
    else:
        topk = float(np.partition(neg_vals, num_neg - k)[num_neg - k :].sum())
    return (pos_sum + topk) / (num_pos + k)


def kernel(output, character_map, affinity_map, character_weight, affinity_weight):
    from concourse.bass_utils import run_bass_kernel_spmd

    global LAST_RESULTS
    np_dt = ml_dtypes.bfloat16

    output = np.asarray(output, dtype=np.float32)

    def shard(a):
        # flat pixel order (b, h, w) -> [core, partition, tile, free]
        return (
            np.ascontiguousarray(a)
            .reshape(N_CORES, P, N_TILES, TW)
            .astype(np_dt)
        )

    packed = np.empty((N_CORES, P, 2, N_TILES, 3, TW), dtype=np_dt)
    packed[:, :, 0, :, 0] = shard(output[:, 0])
    packed[:, :, 0, :, 1] = shard(np.asarray(character_map, dtype=np.float32))
    packed[:, :, 0, :, 2] = shard(np.asarray(character_weight, dtype=np.float32))
    packed[:, :, 1, :, 0] = shard(output[:, 1])
    packed[:, :, 1, :, 1] = shard(np.asarray(affinity_map, dtype=np.float32))
    packed[:, :, 1, :, 2] = shard(np.asarray(affinity_weight, dtype=np.float32))

    in_maps = [{"pk": packed[c]} for c in range(N_CORES)]

    nc = _get_nc()
    res = run_bass_kernel_spmd(
        nc,
        in_maps,
        list(range(N_CORES)),
        trace=os.environ.get("KERNEL_TRACE", "0") == "1",
    )
    LAST_RESULTS = res

    # [cores, PR, branch, col] with col: [s_neg 0:96 | m_pos 96:192 |
    #                                     mw 192:288 | ones 288]
    acc_ps = np.stack([r["acc_ps"] for r in res.results]).astype(np.float64)
    idx = np.arange(BD)
    d0 = acc_ps[:, idx, :, idx].sum(axis=(0, 1))               # <s_neg, l>
    s2 = acc_ps[:, idx, :, 2 * BD + idx].sum(axis=(0, 1))      # <mw, l>
    sum_l = acc_ps[:, 0:BD, :, 3 * BD].sum(axis=(0, 1))        # sum(l)
    row_sneg = acc_ps[:, BD, :, 0:BD].sum(axis=(0, 2))         # 2*num_neg - N
    n_pos = acc_ps[:, BD, :, BD : 2 * BD].sum(axis=(0, 2))     # num_pos

    s1 = (d0 + sum_l) / 2.0
    n_neg = (row_sneg + NPX) / 2.0

    total = 0.0
    for bidx, (tmap, wmap) in enumerate(
        [(character_map, character_weight), (affinity_map, affinity_weight)]
    ):
        num_neg = int(round(n_neg[bidx]))
        num_pos = int(round(n_pos[bidx]))
        k = min(max(1000, 3 * num_pos), num_neg)
        if k == num_neg:
            total += (s1[bidx] + s2[bidx]) / (num_pos + k)
        else:
            # top-k actually selective: fall back to exact host computation
            total += _np_branch_fallback(
                output[:, bidx].reshape(-1),
                np.asarray(tmap, dtype=np.float32).reshape(-1),
                np.asarray(wmap, dtype=np.float32).reshape(-1),
            )

    return np.float32(total)


# revision 19
# speedup vs baseline: 1.0130x; 1.0130x over previous
"""Trainium2 Bass kernel for the CRAFT-style hard-negative-mining MSE loss.

Reference math (per branch, over N = 16*768*768 flat pixels):
    all_loss = (pred - target)^2
    pos_mask = (target >= 0.3) & (weight != 0)
    neg_mask = (target < 0.1)
    pos_sum  = sum(pos_mask * all_loss * weight)
    k        = min(max(1000, 3*num_pos), num_neg)
    topk_sum = sum of k largest all_loss among negatives
    loss     = (pos_sum + topk_sum) / (num_pos + k)
    out      = loss_char + loss_aff

With uniform targets num_pos ~ 0.7*N, so 3*num_pos >> num_neg and
k == num_neg: the top-k degenerates to the full sum over negatives.

Device strategy (v8): per 1/8 shard, per branch-tile [128, W=2304]:
    DVE:  d  = p - t                    tensor_tensor        (2x mode)
          m_pos = (t >= 0.3)           tensor_scalar is_ge  (4x mode)
          mw = m_pos * w                tensor_tensor        (2x mode)
          m_neg = (t < 0.1)            tensor_scalar is_lt  (4x, mask
                                        tiles only)
    ACT:  l  = d^2                      Square               (1x)
          s_neg = sign(0.1 - t) (+-1)   Sign (1x, sign tiles only)
    PE:   24 blocks of 96 data cols:
        psum[0:97,0:385] += [l_96 | 1]^T @ [g0 | g1 | m_pos | mw | one]
          accumulated over the whole branch into one [97, 385] PSUM
          region.

The neg-mask work ALTERNATES tile-by-tile between ScalarE (as a +-1
Sign into group 0) and VectorE (as a 0/1 compare into group 1), so
that DVE, ScalarE, TensorE and DMA all sit just under the ~34us DMA
roofline instead of any one engine being the bottleneck.  The routing
is done by parity of two persistent moving buffers: the sign buffer
has zeros in group 1 and 1.0 in the ones-column (so column 384
accumulates sum(l) over sign tiles, needed to recover S1 from the +-1
encoding); the mask buffer has zeros in group 0 and 0.0 in the
ones-column.

PSUM contents:
    diag rows 0:96, group 0 -> <s_neg, l> = 2*S1_sign - sum_l_sign
    diag rows 0:96, group 1 -> <m_neg, l> = S1_mask
    diag rows 0:96, group 3 -> <mw,    l> = S2
    row 96, group 0         -> sum(s_neg) = 2*n_neg_sign - N/2
    row 96, group 1         -> n_neg_mask
    row 96, group 2         -> n_pos
    col 384 rows 0:96       -> sum(l) over sign tiles
The [97, 385] PSUM regions are copied to SBUF (ScalarE) and DMA'd out
in 4 partition slices over multiple DMA queues; the host recovers
S1/S2/counts, sums across the 8 shards, and applies the k/denominator
logic (with a full numpy fallback for the never-hit-here k < num_neg
case).  Inputs are cast to bf16 on the host: halves HBM traffic and
doubles DVE tensor_tensor throughput.
"""

import os
import numpy as np
import ml_dtypes

N_CORES = 8
B, H, W = 16, 768, 768
NPX = B * H * W              # 9_437_184 flat pixels
P = 128                      # SBUF partitions
FD = NPX // (N_CORES * P)    # 9216 free-dim elements per core per tensor
N_TILES = 4                  # tiles per branch
TW = FD // N_TILES           # 2304 tile width
BD = 96                      # data columns per matmul block
PR = BD + 1                  # psum rows used (96 data + 1 count row)
NB = TW // BD                # 24 matmul blocks per tile
MC = 4 * BD + 2              # moving row: g0 | g1 | m_pos | mw | one | pad
MR = MC - 1                  # columns actually read by the matmul (385)

THRESH_NEG = 0.1
THRESH_POS = 0.3

_compiled = None             # cached nc
LAST_RESULTS = None          # BassKernelResults of the last run (for profiling)


def _build_nc():
    import concourse.bacc as bacc
    import concourse.mybir as mybir
    import concourse.tile as tile
    from contextlib import ExitStack

    DT = mybir.dt.bfloat16
    f32 = mybir.dt.float32
    Alu = mybir.AluOpType
    Act = mybir.ActivationFunctionType

    nc = bacc.Bacc(
        "TRN2",
        target_bir_lowering=False,
        debug=False,
        num_devices=N_CORES,
    )

    # packed input: [P, branch, tile, (p,t,w), TW]
    pk = nc.declare_dram_parameter("pk", [P, 2, N_TILES, 3, TW], DT, isOutput=False)
    out_ps = nc.declare_dram_parameter("acc_ps", [PR, 2, MR], f32, isOutput=True)

    with tile.TileContext(nc) as tc, ExitStack() as ctx:
        in_pool = ctx.enter_context(tc.tile_pool(name="in", bufs=3))
        d_pool = ctx.enter_context(tc.tile_pool(name="d", bufs=2))
        acc_pool = ctx.enter_context(tc.tile_pool(name="acc", bufs=1))
        ps_pool = ctx.enter_context(tc.psum_pool(name="ps", bufs=1))

        ps_sb = acc_pool.tile([PR, 2, MR], f32, tag="ps_sb")
        psum = [
            ps_pool.tile([PR, MR], f32, tag=f"psum{b}", name=f"psum{b}")
            for b in range(2)
        ]
        # per-partition f32 bias constant 0.1 for sign(0.1 - t)
        bias_neg = acc_pool.tile([P, 1], f32, tag="bias_neg")
        nc.gpsimd.memset(bias_neg[:], THRESH_NEG)
        # persistent double-buffered stationary [l_96 | 1] blocks; ones
        # column written once
        lexts = [
            acc_pool.tile([P, NB, PR], DT, tag=f"lext{j}", name=f"lext{j}")
            for j in range(2)
        ]
        # persistent moving buffers: ms[0] for sign tiles, ms[1] for mask
        # tiles; the unused mask group and the ones column are fixed once
        ms = [
            acc_pool.tile([P, NB, MC], DT, tag=f"m{j}", name=f"m{j}")
            for j in range(2)
        ]
        for j in range(2):
            nc.gpsimd.memset(lexts[j][:, :, BD : BD + 1], 1.0)
        nc.gpsimd.memset(ms[0][:, :, BD : 2 * BD], 0.0)          # group 1
        nc.gpsimd.memset(ms[0][:, :, 4 * BD : 4 * BD + 1], 1.0)  # ones col
        nc.gpsimd.memset(ms[1][:, :, 0:BD], 0.0)                 # group 0
        nc.gpsimd.memset(ms[1][:, :, 4 * BD : 4 * BD + 1], 0.0)  # ones col

        it = 0
        for b in range(2):
            for i in range(N_TILES):
                sign_tile = it % 2 == 0
                tin = in_pool.tile([P, 3, TW], DT, tag="in")
                nc.sync.dma_start(tin[:], pk[:, b, i])
                pt = tin[:, 0, :]
                tt = tin[:, 1, :]
                wt = tin[:, 2, :]

                lext = lexts[it % 2]
                m = ms[it % 2]
                if sign_tile:
                    # s_neg = sign(0.1 - t), +-1 exactly    (ACT Sign 1x)
                    nc.scalar.activation(
                        m[:, :, 0:BD], tt, Act.Sign,
                        bias=bias_neg[:], scale=-1.0,
                    )
                # d = pred - target                          (DVE TT 2x)
                d = d_pool.tile([P, TW], DT, tag="d")
                nc.vector.tensor_tensor(d[:], pt, tt, Alu.subtract)
                # l = d^2 into cols 0:96 of the 97-blocks    (ACT Square 1x)
                nc.scalar.activation(lext[:, :, 0:BD], d[:], Act.Square)
                if not sign_tile:
                    # m_neg = (t < 0.1) into group 1         (DVE TS 4x)
                    nc.vector.tensor_scalar(
                        m[:, :, BD : 2 * BD], tt, THRESH_NEG, None, Alu.is_lt
                    )
                # m_pos = (t >= 0.3)                         (DVE TS 4x)
                nc.vector.tensor_scalar(
                    m[:, :, 2 * BD : 3 * BD], tt, THRESH_POS, None, Alu.is_ge
                )
                # mw = m_pos * w                             (DVE TT 2x)
                nc.vector.tensor_tensor(
                    m[:, :, 3 * BD : 4 * BD],
                    m[:, :, 2 * BD : 3 * BD],
                    wt,
                    Alu.mult,
                )

                # psum[b] += [l_blk | 1]^T @ [g0|g1|m_pos|mw|1]       (PE)
                for k in range(NB):
                    nc.tensor.matmul(
                        psum[b][:, :],
                        lext[:, k, :],
                        m[:, k, 0:MR],
                        start=(i == 0 and k == 0),
                        stop=(i == N_TILES - 1 and k == NB - 1),
                    )
                it += 1

            # dump the accumulated [97, 385] PSUM region to SBUF (ScalarE),
            # then DMA it out in 4 partition slices on multiple engine
            # queues (a single contiguous store serializes on one DMA ring)
            nc.scalar.copy(ps_sb[:, b], psum[b][:, :])
            slices = [(0, 25), (25, 49), (49, 73), (73, PR)]
            issuers = [nc.sync, nc.gpsimd, nc.scalar, nc.gpsimd]
            for (p0, p1), eng in zip(slices, issuers):
                eng.dma_start(out_ps[p0:p1, b], ps_sb[p0:p1, b])

    nc.compile()
    return nc


def _get_nc():
    global _compiled
    if _compiled is None:
        _compiled = _build_nc()
    return _compiled


def _np_branch_fallback(pred, target, weight):
    """Exact reference math in numpy float64 (handles k < num_neg)."""
    pred = pred.astype(np.float64)
    target = target.astype(np.float64)
    weight = weight.astype(np.float64)
    all_loss = (pred - target) ** 2
    pos_mask = (target >= THRESH_POS) & (weight != 0)
    neg_mask = target < THRESH_NEG
    pos_sum = float(np.sum(np.where(pos_mask, all_loss * weight, 0.0)))
    num_pos = int(np.sum(pos_mask))
    num_neg = int(np.sum(neg_mask))
    k = min(max(1000, 3 * num_pos), num_neg)
    neg_vals = all_loss[neg_mask]
    if k >= num_neg:
        topk = float(neg_vals.sum())
    elif k <= 0:
        topk = 0.0
    else:
        topk = float(np.partition(neg_vals, num_neg - k)[num_neg - k :].sum())
    return (pos_sum + topk) / (num_pos + k)


def kernel(output, character_map, affinity_map, character_weight, affinity_weight):
    from concourse.bass_utils import run_bass_kernel_spmd

    global LAST_RESULTS
    np_dt = ml_dtypes.bfloat16

    output = np.asarray(output, dtype=np.float32)

    def shard(a):
        # flat pixel order (b, h, w) -> [core, partition, tile, free]
        return (
            np.ascontiguousarray(a)
            .reshape(N_CORES, P, N_TILES, TW)
            .astype(np_dt)
        )

    packed = np.empty((N_CORES, P, 2, N_TILES, 3, TW), dtype=np_dt)
    packed[:, :, 0, :, 0] = shard(output[:, 0])
    packed[:, :, 0, :, 1] = shard(np.asarray(character_map, dtype=np.float32))
    packed[:, :, 0, :, 2] = shard(np.asarray(character_weight, dtype=np.float32))
    packed[:, :, 1, :, 0] = shard(output[:, 1])
    packed[:, :, 1, :, 1] = shard(np.asarray(affinity_map, dtype=np.float32))
    packed[:, :, 1, :, 2] = shard(np.asarray(affinity_weight, dtype=np.float32))

    in_maps = [{"pk": packed[c]} for c in range(N_CORES)]

    nc = _get_nc()
    res = run_bass_kernel_spmd(
        nc,
        in_maps,
        list(range(N_CORES)),
        trace=os.environ.get("KERNEL_TRACE", "0") == "1",
    )
    LAST_RESULTS = res

    # [cores, PR, branch, col], col: [g0 0:96 | g1 96:192 | m_pos 192:288 |
    #                                 mw 288:384 | ones 384]
    acc_ps = np.stack([r["acc_ps"] for r in res.results]).astype(np.float64)
    idx = np.arange(BD)
    d0 = acc_ps[:, idx, :, idx].sum(axis=(0, 1))               # <s_neg, l>
    d1 = acc_ps[:, idx, :, BD + idx].sum(axis=(0, 1))          # S1_mask
    s2 = acc_ps[:, idx, :, 3 * BD + idx].sum(axis=(0, 1))      # <mw, l>
    sum_l_s = acc_ps[:, 0:BD, :, 4 * BD].sum(axis=(0, 1))      # sum_l sign
    r0 = acc_ps[:, BD, :, 0:BD].sum(axis=(0, 2))               # sum(s_neg)
    r1 = acc_ps[:, BD, :, BD : 2 * BD].sum(axis=(0, 2))        # n_neg_mask
    n_pos = acc_ps[:, BD, :, 2 * BD : 3 * BD].sum(axis=(0, 2))  # num_pos

    # per branch, sign tiles cover exactly half the branch's pixels
    n_sign = NPX / 2.0
    s1 = (d0 + sum_l_s) / 2.0 + d1
    n_neg = (r0 + n_sign) / 2.0 + r1

    total = 0.0
    for bidx, (tmap, wmap) in enumerate(
        [(character_map, character_weight), (affinity_map, affinity_weight)]
    ):
        num_neg = int(round(n_neg[bidx]))
        num_pos = int(round(n_pos[bidx]))
        k = min(max(1000, 3 * num_pos), num_neg)
        if k == num_neg:
            total += (s1[bidx] + s2[bidx]) / (num_pos + k)
        else:
            # top-k actually selective: fall back to exact host computation
            total += _np_branch_fallback(
                output[:, bidx].reshape(-1),
                np.asarray(tmap, dtype=np.float32).reshape(-1),
                np.asarray(wmap, dtype=np.float32).reshape(-1),
            )

    return np.float32(total)


# revision 20
# speedup vs baseline: 1.0212x; 1.0081x over previous
"""Trainium2 Bass kernel for the CRAFT-style hard-negative-mining MSE loss.

Reference math (per branch, over N = 16*768*768 flat pixels):
    all_loss = (pred - target)^2
    pos_mask = (target >= 0.3) & (weight != 0)
    neg_mask = (target < 0.1)
    pos_sum  = sum(pos_mask * all_loss * weight)
    k        = min(max(1000, 3*num_pos), num_neg)
    topk_sum = sum of k largest all_loss among negatives
    loss     = (pos_sum + topk_sum) / (num_pos + k)
    out      = loss_char + loss_aff

With uniform targets num_pos ~ 0.7*N, so 3*num_pos >> num_neg and
k == num_neg: the top-k degenerates to the full sum over negatives.

Device strategy (v8): per 1/8 shard, per branch-tile [128, W=2304]:
    DVE:  d  = p - t                    tensor_tensor        (2x mode)
          m_pos = (t >= 0.3)           tensor_scalar is_ge  (4x mode)
          mw = m_pos * w                tensor_tensor        (2x mode)
          m_neg = (t < 0.1)            tensor_scalar is_lt  (4x, mask
                                        tiles only)
    ACT:  l  = d^2                      Square               (1x)
          s_neg = sign(0.1 - t) (+-1)   Sign (1x, sign tiles only)
    PE:   24 blocks of 96 data cols:
        psum[0:97,0:385] += [l_96 | 1]^T @ [g0 | g1 | m_pos | mw | one]
          accumulated over the whole branch into one [97, 385] PSUM
          region.

The neg-mask work ALTERNATES tile-by-tile between ScalarE (as a +-1
Sign into group 0) and VectorE (as a 0/1 compare into group 1), so
that DVE, ScalarE, TensorE and DMA all sit just under the ~34us DMA
roofline instead of any one engine being the bottleneck.  The routing
is done by parity of two persistent moving buffers: the sign buffer
has zeros in group 1 and 1.0 in the ones-column (so column 384
accumulates sum(l) over sign tiles, needed to recover S1 from the +-1
encoding); the mask buffer has zeros in group 0 and 0.0 in the
ones-column.

PSUM contents:
    diag rows 0:96, group 0 -> <s_neg, l> = 2*S1_sign - sum_l_sign
    diag rows 0:96, group 1 -> <m_neg, l> = S1_mask
    diag rows 0:96, group 3 -> <mw,    l> = S2
    row 96, group 0         -> sum(s_neg) = 2*n_neg_sign - N/2
    row 96, group 1         -> n_neg_mask
    row 96, group 2         -> n_pos
    col 384 rows 0:96       -> sum(l) over sign tiles
The [97, 385] PSUM regions are copied to SBUF (ScalarE) and DMA'd out
in 4 partition slices over multiple DMA queues; the host recovers
S1/S2/counts, sums across the 8 shards, and applies the k/denominator
logic (with a full numpy fallback for the never-hit-here k < num_neg
case).  Inputs are cast to bf16 on the host: halves HBM traffic and
doubles DVE tensor_tensor throughput.
"""

import os
import numpy as np
import ml_dtypes

N_CORES = 8
B, H, W = 16, 768, 768
NPX = B * H * W              # 9_437_184 flat pixels
P = 128                      # SBUF partitions
FD = NPX // (N_CORES * P)    # 9216 free-dim elements per core per tensor
N_TILES = 4                  # tiles per branch
TW = FD // N_TILES           # 2304 tile width
BD = 96                      # data columns per matmul block
PR = BD + 1                  # psum rows used (96 data + 1 count row)
NB = TW // BD                # 24 matmul blocks per tile
MC = 4 * BD + 2              # moving row: g0 | g1 | m_pos | mw | one | pad
MR = MC - 1                  # columns actually read by the matmul (385)

THRESH_NEG = 0.1
THRESH_POS = 0.3

_compiled = None             # cached nc
LAST_RESULTS = None          # BassKernelResults of the last run (for profiling)


def _build_nc():
    import concourse.bacc as bacc
    import concourse.mybir as mybir
    import concourse.tile as tile
    from contextlib import ExitStack

    DT = mybir.dt.bfloat16
    f32 = mybir.dt.float32
    Alu = mybir.AluOpType
    Act = mybir.ActivationFunctionType

    nc = bacc.Bacc(
        "TRN2",
        target_bir_lowering=False,
        debug=False,
        num_devices=N_CORES,
    )

    # packed input: [P, branch, tile, (p,t,w), TW]
    pk = nc.declare_dram_parameter("pk", [P, 2, N_TILES, 3, TW], DT, isOutput=False)
    out_ps = nc.declare_dram_parameter("acc_ps", [PR, 2, MR], f32, isOutput=True)

    with tile.TileContext(nc) as tc, ExitStack() as ctx:
        in_pool = ctx.enter_context(tc.tile_pool(name="in", bufs=3))
        d_pool = ctx.enter_context(tc.tile_pool(name="d", bufs=2))
        acc_pool = ctx.enter_context(tc.tile_pool(name="acc", bufs=1))
        ps_pool = ctx.enter_context(tc.psum_pool(name="ps", bufs=1))

        ps_sb = acc_pool.tile([PR, 2, MR], f32, tag="ps_sb")
        psum = [
            ps_pool.tile([PR, MR], f32, tag=f"psum{b}", name=f"psum{b}")
            for b in range(2)
        ]
        # per-partition f32 bias constant 0.1 for sign(0.1 - t)
        bias_neg = acc_pool.tile([P, 1], f32, tag="bias_neg")
        nc.gpsimd.memset(bias_neg[:], THRESH_NEG)
        # persistent double-buffered stationary [l_96 | 1] blocks; ones
        # column written once
        lexts = [
            acc_pool.tile([P, NB, PR], DT, tag=f"lext{j}", name=f"lext{j}")
            for j in range(2)
        ]
        # persistent moving buffers: ms[0] for sign tiles, ms[1] for mask
        # tiles; the unused mask group and the ones column are fixed once
        ms = [
            acc_pool.tile([P, NB, MC], DT, tag=f"m{j}", name=f"m{j}")
            for j in range(2)
        ]
        for j in range(2):
            nc.gpsimd.memset(lexts[j][:, :, BD : BD + 1], 1.0)
        nc.gpsimd.memset(ms[0][:, :, BD : 2 * BD], 0.0)          # group 1
        nc.gpsimd.memset(ms[0][:, :, 4 * BD : 4 * BD + 1], 1.0)  # ones col
        nc.gpsimd.memset(ms[1][:, :, 0:BD], 0.0)                 # group 0
        nc.gpsimd.memset(ms[1][:, :, 4 * BD : 4 * BD + 1], 0.0)  # ones col

        # PE warm-up: the HAM clock gate keeps TensorE at 1.2 GHz until it
        # has been busy ~3.4us.  The first real matmul only lands ~15us in
        # (after DMA+DVE+ACT of tile 0), so burn the idle head on dummy
        # matmuls into a scratch PSUM bank to reach 2.4 GHz before the
        # real stream starts.
        ps_warm = ps_pool.tile([P, 512], f32, tag="ps_warm")
        nc.gpsimd.memset(ms[1][:, 0, 0:MC], 0.0)
        for _ in range(34):
            nc.tensor.matmul(
                ps_warm[:, 0:MR],
                ms[1][:, 0, 0:128],
                ms[1][:, 0, 0:MR],
                start=True,
                stop=True,
            )

        it = 0
        for b in range(2):
            for i in range(N_TILES):
                sign_tile = it % 2 == 0
                tin = in_pool.tile([P, 3, TW], DT, tag="in")
                nc.sync.dma_start(tin[:], pk[:, b, i])
                pt = tin[:, 0, :]
                tt = tin[:, 1, :]
                wt = tin[:, 2, :]

                lext = lexts[it % 2]
                m = ms[it % 2]
                if sign_tile:
                    # s_neg = sign(0.1 - t), +-1 exactly    (ACT Sign 1x)
                    nc.scalar.activation(
                        m[:, :, 0:BD], tt, Act.Sign,
                        bias=bias_neg[:], scale=-1.0,
                    )
                # d = pred - target                          (DVE TT 2x)
                d = d_pool.tile([P, TW], DT, tag="d")
                nc.vector.tensor_tensor(d[:], pt, tt, Alu.subtract)
                # l = d^2 into cols 0:96 of the 97-blocks    (ACT Square 1x)
                nc.scalar.activation(lext[:, :, 0:BD], d[:], Act.Square)
                if not sign_tile:
                    # m_neg = (t < 0.1) into group 1         (DVE TS 4x)
                    nc.vector.tensor_scalar(
                        m[:, :, BD : 2 * BD], tt, THRESH_NEG, None, Alu.is_lt
                    )
                # m_pos = (t >= 0.3)                         (DVE TS 4x)
                nc.vector.tensor_scalar(
                    m[:, :, 2 * BD : 3 * BD], tt, THRESH_POS, None, Alu.is_ge
                )
                # mw = m_pos * w                             (DVE TT 2x)
                nc.vector.tensor_tensor(
                    m[:, :, 3 * BD : 4 * BD],
                    m[:, :, 2 * BD : 3 * BD],
                    wt,
                    Alu.mult,
                )

                # psum[b] += [l_blk | 1]^T @ [g0|g1|m_pos|mw|1]       (PE)
                for k in range(NB):
                    nc.tensor.matmul(
                        psum[b][:, :],
                        lext[:, k, :],
                        m[:, k, 0:MR],
                        start=(i == 0 and k == 0),
                        stop=(i == N_TILES - 1 and k == NB - 1),
                    )
                it += 1

            # dump the accumulated [97, 385] PSUM region to SBUF (ScalarE),
            # then DMA it out in 4 partition slices on multiple engine
            # queues (a single contiguous store serializes on one DMA ring)
            nc.scalar.copy(ps_sb[:, b], psum[b][:, :])
            slices = [(0, 25), (25, 49), (49, 73), (73, PR)]
            issuers = [nc.sync, nc.gpsimd, nc.scalar, nc.gpsimd]
            for (p0, p1), eng in zip(slices, issuers):
                eng.dma_start(out_ps[p0:p1, b], ps_sb[p0:p1, b])

    nc.compile()
    return nc


def _get_nc():
    global _compiled
    if _compiled is None:
        _compiled = _build_nc()
    return _compiled


def _np_branch_fallback(pred, target, weight):
    """Exact reference math in numpy float64 (handles k < num_neg)."""
    pred = pred.astype(np.float64)
    target = target.astype(np.float64)
    weight = weight.astype(np.float64)
    all_loss = (pred - target) ** 2
    pos_mask = (target >= THRESH_POS) & (weight != 0)
    neg_mask = target < THRESH_NEG
    pos_sum = float(np.sum(np.where(pos_mask, all_loss * weight, 0.0)))
    num_pos = int(np.sum(pos_mask))
    num_neg = int(np.sum(neg_mask))
    k = min(max(1000, 3 * num_pos), num_neg)
    neg_vals = all_loss[neg_mask]
    if k >= num_neg:
        topk = float(neg_vals.sum())
    elif k <= 0:
        topk = 0.0
    else:
        topk = float(np.partition(neg_vals, num_neg - k)[num_neg - k :].sum())
    return (pos_sum + topk) / (num_pos + k)


def kernel(output, character_map, affinity_map, character_weight, affinity_weight):
    from concourse.bass_utils import run_bass_kernel_spmd

    global LAST_RESULTS
    np_dt = ml_dtypes.bfloat16

    output = np.asarray(output, dtype=np.float32)

    def shard(a):
        # flat pixel order (b, h, w) -> [core, partition, tile, free]
        return (
            np.ascontiguousarray(a)
            .reshape(N_CORES, P, N_TILES, TW)
            .astype(np_dt)
        )

    packed = np.empty((N_CORES, P, 2, N_TILES, 3, TW), dtype=np_dt)
    packed[:, :, 0, :, 0] = shard(output[:, 0])
    packed[:, :, 0, :, 1] = shard(np.asarray(character_map, dtype=np.float32))
    packed[:, :, 0, :, 2] = shard(np.asarray(character_weight, dtype=np.float32))
    packed[:, :, 1, :, 0] = shard(output[:, 1])
    packed[:, :, 1, :, 1] = shard(np.asarray(affinity_map, dtype=np.float32))
    packed[:, :, 1, :, 2] = shard(np.asarray(affinity_weight, dtype=np.float32))

    in_maps = [{"pk": packed[c]} for c in range(N_CORES)]

    nc = _get_nc()
    res = run_bass_kernel_spmd(
        nc,
        in_maps,
        list(range(N_CORES)),
        trace=os.environ.get("KERNEL_TRACE", "0") == "1",
    )
    LAST_RESULTS = res

    # [cores, PR, branch, col], col: [g0 0:96 | g1 96:192 | m_pos 192:288 |
    #                                 mw 288:384 | ones 384]
    acc_ps = np.stack([r["acc_ps"] for r in res.results]).astype(np.float64)
    idx = np.arange(BD)
    d0 = acc_ps[:, idx, :, idx].sum(axis=(0, 1))               # <s_neg, l>
    d1 = acc_ps[:, idx, :, BD + idx].sum(axis=(0, 1))          # S1_mask
    s2 = acc_ps[:, idx, :, 3 * BD + idx].sum(axis=(0, 1))      # <mw, l>
    sum_l_s = acc_ps[:, 0:BD, :, 4 * BD].sum(axis=(0, 1))      # sum_l sign
    r0 = acc_ps[:, BD, :, 0:BD].sum(axis=(0, 2))               # sum(s_neg)
    r1 = acc_ps[:, BD, :, BD : 2 * BD].sum(axis=(0, 2))        # n_neg_mask
    n_pos = acc_ps[:, BD, :, 2 * BD : 3 * BD].sum(axis=(0, 2))  # num_pos

    # per branch, sign tiles cover exactly half the branch's pixels
    n_sign = NPX / 2.0
    s1 = (d0 + sum_l_s) / 2.0 + d1
    n_neg = (r0 + n_sign) / 2.0 + r1

    total = 0.0
    for bidx, (tmap, wmap) in enumerate(
        [(character_map, character_weight), (affinity_map, affinity_weight)]
    ):
        num_neg = int(round(n_neg[bidx]))
        num_pos = int(round(n_pos[bidx]))
        k = min(max(1000, 3 * num_pos), num_neg)
        if k == num_neg:
            total += (s1[bidx] + s2[bidx]) / (num_pos + k)
        else:
            # top-k actually selective: fall back to exact host computation
            total += _np_branch_fallback(
                output[:, bidx].reshape(-1),
                np.asarray(tmap, dtype=np.float32).reshape(-1),
                np.asarray(wmap, dtype=np.float32).reshape(-1),
            )

    return np.float32(total)
